# revision 2
# baseline (speedup 1.0000x reference)
"""Causal self-attention Trainium2 kernel, v2.

Problem: B=8, T=1024, C=768, H=12 heads, D=64. fp32 in/out.
Data-parallel over batch: each of 8 NeuronCores does one batch element.

v2 structure (vs v1):
  - All matmul operands are bf16, pre-cast and pre-packed HOST-side into the
    exact SBUF layouts (halves HBM traffic, removes every on-chip cast).
  - Attention processes HEAD PAIRS: the two heads of qkT tile mq live at
    partitions 0:64 / 64:128, and their K=64 wei matmuls are emitted
    back-to-back at tile_position (0,0)/(64,0) -> concurrent row-groups on HW.
  - Queries processed in 512-col halves so every attention PSUM tile is one
    bank: wei tag 3 bufs + outT A/B 2 bufs each + qk filler 1 buf = 8 banks.
  - The NEXT pair's qkv projection matmuls are interleaved into the attention
    j-loop as PE filler, so the ACT-paced exp never leaves the PE idle.
  - softmax denominators accumulate free in PSUM row 64 via a ones-column in
    V ([v | 1] stationary); normalization = DVE reciprocal of row 64 +
    gpsimd partition_broadcast + one DVE multiply per head-half.
  - ACT does exp only; qkT PSUM->SBUF moves (with fused qkv bias) are DVE
    tensor_scalar_add; causal diag masks are gpsimd tensor_mul off-path.
"""

import numpy as np
from contextlib import ExitStack

import bass_rust
import ml_dtypes
import concourse.bass as bass
import concourse.tile as tile
from concourse import mybir
from concourse.bass_utils import run_bass_kernel_spmd

F32 = mybir.dt.float32
BF16 = mybir.dt.bfloat16
AF = mybir.ActivationFunctionType

B, T, C = 8, 1024, 768
H, D = 12, 64
NT = T // 128    # 8 key blocks
KC = C // 128    # 6 contraction chunks
NPAIR = H // 2   # 6 head pairs; qkT m-tile mq holds heads (2mq, 2mq+1)
MQK = 2 * NPAIR  # 12 m-tiles over q,k columns


def _patched_drain_and_barrier(self, tick_clock, wait_clock):
    # Walrus in this environment rejects >1 sync-wait on a single SP drain
    # ("Too many sync wait commands"); split the tail waits across a chain
    # of drains carrying one wait each.
    nc_ = self.nc
    drain_inst = nc_.sync.drain()
    wait_clock.add_sem_waits(
        drain_inst.ins, bass_rust.ScopedClock({None: tick_clock.global_clock})
    )
    si = drain_inst.ins.sync_info
    waits = list(si.on_wait or [])
    if len(waits) > 1:
        si.on_wait = waits[:1]
        for i in range(1, len(waits)):
            extra = nc_.sync.drain()
            extra.ins.sync_info = bass_rust.SyncInfo(
                on_wait=waits[i : i + 1], on_update=[]
            )
    nc_.all_engine_barrier()
    popped = nc_._tile_sem_poison_stack.pop()
    assert popped is self._sem_poison
    nc_.clear_and_free_semaphores(list(self.sems.allocated().values()))
    nc_.all_engine_barrier()


tile.TileContext._drain_and_barrier = _patched_drain_and_barrier


def _split_multi_waits(nc, max_waits=1):
    """Walrus here allows only `max_waits` sync-wait commands per instruction.
    Hoist excess waits onto standalone EventSemaphore ops inserted just before
    the owning instruction on the same engine (same blocking semantics)."""
    n_new = 0
    for fn in nc.m.functions:
        for blk in fn.blocks:
            insts = blk.instructions
            out = []
            for inst in insts:
                si = getattr(inst, "sync_info", None)
                waits = list(si.on_wait) if si and si.on_wait else []
                if len(waits) > max_waits:
                    keep = waits[-max_waits:]
                    hoist = waits[: -max_waits]
                    for w in hoist:
                        ev = mybir.InstEventSemaphore(
                            name=f"Wsplit-{nc.next_id()}", ins=[], outs=[]
                        )
                        ev.engine = inst.engine
                        ev.sync_info = bass_rust.SyncInfo(
                            on_wait=[w], on_update=[]
                        )
                        nc.inst_map[ev.name] = ev
                        out.append(ev)
                        n_new += 1
                    si.on_wait = keep
                out.append(inst)
            if n_new:
                insts[:] = out
    return n_new


def _emit_rep(nc, tc, aps):
    xT, waqk, wav, wp, baqk, bv, bp, mask01, ones, y = aps
    with ExitStack() as ctx:
        consts = ctx.enter_context(tc.tile_pool(name="consts", bufs=1))
        loads = ctx.enter_context(tc.tile_pool(name="loads", bufs=1))
        qkT_pool = ctx.enter_context(tc.tile_pool(name="qkTp", bufs=1))
        v_pool = ctx.enter_context(tc.tile_pool(name="vp", bufs=1))
        att_pool = ctx.enter_context(tc.tile_pool(name="attp", bufs=1))
        work = ctx.enter_context(tc.tile_pool(name="work", bufs=1))

        # ---- constants + weight/x loads (all pre-packed bf16 host-side) ----
        baqk_sb = consts.tile([128, MQK], F32, name="baqk_sb")
        nc.sync.dma_start(baqk_sb[:], baqk[:])
        mask_sb = consts.tile([128, 128], BF16, name="mask_sb")
        nc.sync.dma_start(mask_sb[:], mask01[:])
        bv_sb = consts.tile([128, C], F32, name="bv_sb")
        nc.sync.dma_start(bv_sb[:], bv[:])
        ones_sb = consts.tile([128, 64], mybir.dt.float32r, name="ones_sb")
        nc.sync.dma_start(ones_sb[:], ones[:])

        # warm the ACT exp table during the load phase (the first real exp
        # would otherwise pay the ~2us table load mid-pipeline)
        warm = consts.tile([128, 1], F32, name="warm")
        nc.scalar.activation(warm[0:1, :], baqk_sb[0:1, 0:1], AF.Exp)

        xT_sb = [None] * KC
        waqk_sb = [None] * MQK

        def load_xt(kc):
            t = loads.tile([128, T], BF16, tag=f"xT{kc}", name=f"xT_sb{kc}")
            nc.sync.dma_start(t[:], xT[:, kc, :])
            xT_sb[kc] = t

        def load_wa(m):
            t = loads.tile([128, KC, 128], BF16, tag=f"wa{m}", name=f"wa_sb{m}")
            nc.sync.dma_start(t[:], waqk[:, m])
            waqk_sb[m] = t

        # first qk matmul (m=0, kc=0) needs only xT[0] + waqk[0]
        load_xt(0)
        load_wa(0)
        load_wa(NPAIR)
        for kc in range(1, KC):
            load_xt(kc)
        wav_sb = loads.tile([128, KC, C], BF16, name="wav_sb")
        nc.sync.dma_start(wav_sb[:], wav[:])
        for mq in range(1, NPAIR):
            load_wa(mq)
            load_wa(NPAIR + mq)
        wp_sb = loads.tile([128, KC, C], BF16, name="wp_sb")
        nc.sync.dma_start(wp_sb[:], wp[:])
        bp_sb = consts.tile([128, C], F32, name="bp_sb")
        nc.sync.dma_start(bp_sb[:], bp[:])

        qkT = [
            qkT_pool.tile([128, T], BF16, tag=f"qkT{m}", name=f"qkT{m}")
            for m in range(MQK)
        ]
        v_sb = v_pool.tile([128, NT, H, 65], BF16, name="v_sb")
        nc.vector.memset(v_sb[:, :, :, 64], 1.0)
        attT = [
            att_pool.tile([128, T], BF16, tag=f"attT{kc}", name=f"attT{kc}")
            for kc in range(KC)
        ]

        def qk_pair_gen(mq, ps_pool, bufs):
            """Emit one instruction per next(): the qkv q/k projections for
            pair mq (m-tiles mq and NPAIR+mq) in 512-col halves."""
            for m in (mq, NPAIR + mq):
                for half in (0, 1):
                    qk_ps = ps_pool.tile(
                        [128, 512], F32, tag="qk", bufs=bufs,
                        name=f"qk_ps{m}_{half}",
                    )
                    for kc in range(KC):
                        nc.tensor.matmul(
                            qk_ps[:],
                            waqk_sb[m][:, kc, :],
                            xT_sb[kc][:, half * 512 : (half + 1) * 512],
                            start=(kc == 0),
                            stop=(kc == KC - 1),
                        )
                        yield
                    nc.vector.tensor_scalar_add(
                        qkT[m][:, half * 512 : (half + 1) * 512],
                        qk_ps[:],
                        baqk_sb[:, m : m + 1],
                    )
                    yield

        # ---- scope 1: qk projections for pair 0, then V ----
        with tc.tile_pool(name="ps1", bufs=1, space="PSUM") as ps1:
            for _ in qk_pair_gen(0, ps1, bufs=2):
                pass
            for tt in range(NT):
                v_ps = ps1.tile([128, C], F32, tag="v", bufs=2, name=f"v_ps{tt}")
                for kc in range(KC):
                    lhsT = xT_sb[kc][:, tt * 128 : (tt + 1) * 128]
                    nc.tensor.matmul(
                        v_ps[:, 0:512], lhsT, wav_sb[:, kc, 0:512],
                        start=(kc == 0), stop=(kc == KC - 1),
                    )
                    nc.tensor.matmul(
                        v_ps[:, 512:C], lhsT, wav_sb[:, kc, 512:C],
                        start=(kc == 0), stop=(kc == KC - 1),
                    )
                nc.vector.tensor_add(
                    v_sb[:, tt, :, 0:64],
                    v_ps.rearrange("p (h d) -> p h d", h=H),
                    bv_sb.rearrange("p (h d) -> p h d", h=H),
                )

        # ---- scope 2: attention per head pair, next pair's qk interleaved ----
        with tc.tile_pool(name="ps2", bufs=1, space="PSUM") as ps2:
            for mq in range(NPAIR):
                filler = (
                    qk_pair_gen(mq + 1, ps2, bufs=1)
                    if mq + 1 < NPAIR
                    else iter(())
                )

                def fill(n):
                    for _ in range(n):
                        if next(filler, "done") == "done":
                            return

                qA = qkT[mq]
                kA = qkT[NPAIR + mq]
                for half in (0, 1):
                    Qlo = half * 512
                    jmax = 4 * (half + 1) - 1
                    outT = [
                        ps2.tile([128, 512], F32, tag="outTA", bufs=2,
                                 name=f"outTA{mq}_{half}"),
                        ps2.tile([128, 512], F32, tag="outTB", bufs=2,
                                 name=f"outTB{mq}_{half}"),
                    ]

                    def emit_pv(j, w, pT):
                        for par in (0, 1):
                            nc.tensor.matmul(
                                outT[par][0:65, w:512],
                                v_sb[:, j, 2 * mq + par, 0:65],
                                pT[:, par * 512 + w : (par + 1) * 512],
                                start=(j == 0),
                                stop=(j == jmax),
                                skip_group_check=True,
                            )

                    pend = None
                    for j in range(jmax + 1):
                        t_lo = j * 128
                        w = max(t_lo - Qlo, 0)  # first valid col within half
                        wei = []
                        for par in (0, 1):
                            wps = ps2.tile(
                                [128, 512], F32, tag="wei", bufs=2,
                                name=f"wei{mq}_{half}_{j}_{par}",
                            )
                            # K=64 matmul; two heads at disjoint row groups
                            # run concurrently on HW
                            nc.tensor.matmul(
                                wps[:, w:512],
                                kA[par * 64 : (par + 1) * 64, t_lo : t_lo + 128],
                                qA[par * 64 : (par + 1) * 64, Qlo + w : Qlo + 512],
                                start=True,
                                stop=True,
                                tile_position=(par * 64, 0),
                            )
                            wei.append(wps)
                        pT = work.tile(
                            [128, 1024], BF16, tag="pT", bufs=3,
                            name=f"pT{mq}_{half}_{j}",
                        )
                        for par in (0, 1):
                            nc.scalar.activation(
                                pT[:, par * 512 + w : (par + 1) * 512],
                                wei[par][:, w:512],
                                AF.Exp,
                            )
                        if t_lo >= Qlo:
                            # zero the invalid triangle of the diagonal chunk
                            for par in (0, 1):
                                nc.gpsimd.tensor_mul(
                                    pT[:, par * 512 + w : par * 512 + w + 128],
                                    pT[:, par * 512 + w : par * 512 + w + 128],
                                    mask_sb[:],
                                )
                        if pend is not None:
                            emit_pv(*pend)
                        fill(2)
                        pend = (j, w, pT)
                    emit_pv(*pend)

                    # normalization: att^T = outT rows 0:64 * (1/S), S = row 64
                    for par in (0, 1):
                        recS = work.tile(
                            [128, 512], mybir.dt.float32r, tag="recS", bufs=2,
                            name=f"recS{mq}_{half}_{par}",
                        )
                        nc.vector.reciprocal(recS[64:65, :], outT[par][64:65, :])
                        # broadcast 1/S across 64 partitions via rank-1 matmul
                        # (stationary ones [1,64] at PE row 64), stage to SBUF
                        # with a PSUM->SBUF DMA (keeps ACT/DVE off this path)
                        recB_ps = ps2.tile(
                            [128, 512], F32, tag="recB", bufs=1,
                            name=f"recB_ps{mq}_{half}_{par}",
                        )
                        nc.tensor.matmul(
                            recB_ps[0:64, :],
                            ones_sb[64:65, :],
                            recS[64:65, :],
                            start=True,
                            stop=True,
                            tile_position=(64, 0),
                        )
                        recB = work.tile(
                            [128, 512], F32, tag="recB", bufs=2,
                            name=f"recB{mq}_{half}_{par}",
                        )
                        if par == 0:
                            # ACT: fast path so the par=1 broadcast matmul
                            # (which reuses the recB PSUM slot) is not
                            # queue-blocked behind DVE norm work
                            nc.scalar.activation(
                                recB[0:64, :], recB_ps[0:64, :], AF.Copy
                            )
                        else:
                            nc.vector.tensor_copy(recB[0:64, :], recB_ps[0:64, :])
                        if par == 0:
                            nc.vector.tensor_mul(
                                attT[mq][0:64, Qlo : Qlo + 512],
                                outT[0][0:64, :],
                                recB[0:64, :],
                            )
                        else:
                            # DVE is lane-locked: compute at rows 0:64, DMA
                            # the partition shift into attT rows 64:128
                            shf = work.tile(
                                [128, 512], BF16, tag="shift", bufs=2,
                                name=f"shift{mq}_{half}",
                            )
                            nc.vector.tensor_mul(
                                shf[0:64, :], outT[1][0:64, :], recB[0:64, :]
                            )
                            nc.sync.dma_start(
                                attT[mq][64:128, Qlo : Qlo + 512], shf[0:64, :]
                            )
                        fill(2)
                    fill(2)
                for _ in filler:
                    pass

        # ---- scope 3: output projection ----
        with (
            tc.tile_pool(name="ps3", bufs=1, space="PSUM") as ps3,
            tc.tile_pool(name="yout", bufs=3) as yout,
        ):
            for tt in range(NT):
                y_ps = ps3.tile([128, C], F32, tag="y", bufs=2, name=f"y_ps{tt}")
                for kc in range(KC):
                    lhsT = attT[kc][:, tt * 128 : (tt + 1) * 128]
                    nc.tensor.matmul(
                        y_ps[:, 0:512], lhsT, wp_sb[:, kc, 0:512],
                        start=(kc == 0), stop=(kc == KC - 1),
                    )
                    nc.tensor.matmul(
                        y_ps[:, 512:C], lhsT, wp_sb[:, kc, 512:C],
                        start=(kc == 0), stop=(kc == KC - 1),
                    )
                y_sb = yout.tile([128, C], F32, tag="ysb", name=f"y_sb{tt}")
                nc.vector.tensor_add(y_sb[:], y_ps[:], bp_sb[:])
                nc.sync.dma_start(y[tt * 128 : (tt + 1) * 128, :], y_sb[:])


def build_attention_kernel(reps=1):
    nc = bass.Bass("TRN2", target_bir_lowering=False, debug=False)

    xT = nc.dram_tensor("xT", [128, KC, T], BF16, kind="ExternalInput").ap()
    waqk = nc.dram_tensor(
        "waqk", [128, MQK, KC, 128], BF16, kind="ExternalInput"
    ).ap()
    wav = nc.dram_tensor("wav", [128, KC, C], BF16, kind="ExternalInput").ap()
    wp = nc.dram_tensor("wp", [128, KC, C], BF16, kind="ExternalInput").ap()
    baqk = nc.dram_tensor("baqk", [128, MQK], F32, kind="ExternalInput").ap()
    bv = nc.dram_tensor("bv", [128, C], F32, kind="ExternalInput").ap()
    bp = nc.dram_tensor("bp", [128, C], F32, kind="ExternalInput").ap()
    mask01 = nc.dram_tensor("mask01", [128, 128], BF16, kind="ExternalInput").ap()
    ones = nc.dram_tensor("ones", [128, 64], mybir.dt.float32r, kind="ExternalInput").ap()
    y = nc.dram_tensor("y", [T, C], F32, kind="ExternalOutput").ap()
    aps = (xT, waqk, wav, wp, baqk, bv, bp, mask01, ones, y)

    with tile.TileContext(nc) as tc:
        with nc.allow_low_precision(reason="bf16 matmul operands"):
            for _ in range(reps):
                _emit_rep(nc, tc, aps)

    _split_multi_waits(nc)
    return nc


_NC_CACHE = None


def _get_nc():
    global _NC_CACHE
    if _NC_CACHE is None:
        _NC_CACHE = build_attention_kernel()
    return _NC_CACHE


def make_in_maps(x, w_attn, b_attn, w_proj, b_proj):
    bf16 = ml_dtypes.bfloat16
    x = np.asarray(x, dtype=np.float32)
    w_attn = np.asarray(w_attn, dtype=np.float32)
    b_attn = np.asarray(b_attn, dtype=np.float32)
    w_proj = np.asarray(w_proj, dtype=np.float32)
    b_proj = np.asarray(b_proj, dtype=np.float32)

    # [128, MQK, KC, 128]: waqk[p, m, kc, n] = w_attn[kc*128+p, m*128+n]
    waqk = np.ascontiguousarray(
        w_attn[:, : 2 * C].reshape(KC, 128, MQK, 128).transpose(1, 2, 0, 3)
    ).astype(bf16)
    # [128, KC, C]: wav[p, kc, n] = w_attn[kc*128+p, 2C+n]
    wav = np.ascontiguousarray(
        w_attn[:, 2 * C :].reshape(KC, 128, C).transpose(1, 0, 2)
    ).astype(bf16)
    wpb = np.ascontiguousarray(
        w_proj.reshape(KC, 128, C).transpose(1, 0, 2)
    ).astype(bf16)
    baqk = np.ascontiguousarray(b_attn[: 2 * C].reshape(MQK, 128).T)
    bv = np.ascontiguousarray(np.broadcast_to(b_attn[2 * C :], (128, C)))
    bp = np.ascontiguousarray(np.broadcast_to(b_proj, (128, C)))
    sl, tl = np.meshgrid(np.arange(128), np.arange(128), indexing="ij")
    mask01 = (tl >= sl).astype(bf16)
    ones64 = np.ones((128, 64), dtype=np.float32)

    in_maps = []
    for b in range(B):
        xTb = np.ascontiguousarray(
            x[b].T.reshape(KC, 128, T).transpose(1, 0, 2)
        ).astype(bf16)
        in_maps.append(
            {
                "xT": xTb,
                "waqk": waqk,
                "wav": wav,
                "wp": wpb,
                "baqk": baqk,
                "bv": bv,
                "bp": bp,
                "mask01": mask01,
                "ones": ones64,
            }
        )
    return in_maps


def kernel(x, w_attn, b_attn, w_proj, b_proj):
    nc = _get_nc()
    in_maps = make_in_maps(x, w_attn, b_attn, w_proj, b_proj)
    res = run_bass_kernel_spmd(nc, in_maps, core_ids=list(range(B)))
    return np.stack([res.results[i]["y"] for i in range(B)], axis=0)
